# revision 6
# baseline (speedup 1.0000x reference)
"""EulerAttention Trainium2 kernel.

Per-core sharding: core c in 0..7 -> (batch b = c // 4, query block qb = c % 4,
1024 queries each).  Each core computes K/V (+ feature maps) for its whole
batch, Q features for its query block, then flash-style scores/softmax/AV.

All matmuls run as float32r (full-rate fp32 with reduced multiplier mantissa).
Feature maps cos/sin(theta) are computed with a turns-space range reduction
(magic-number round + add_range_wrap) feeding the ACT Sin table (valid +-pi).

kernel(**inputs) takes the full unsharded inputs from reference.setup_inputs()
and returns the full [B, S, D] output.
"""
import sys, math

sys.path.insert(0, "/opt/trn_rl_repo")

import numpy as np

B, S, D = 2, 4096, 1024
NCORES = 8
QBLK = S // 4          # queries per core
ET = D // 128          # number of 128-row e/d tiles (8)
MAGIC = float(1.5 * 2**23)
TWOPI = 2.0 * math.pi
INV_SQRT_D = 1.0 / math.sqrt(D)

_cache = {}


def _build_program(s_keys=S, s_q=QBLK):
    """Build the SPMD bass program. s_keys/s_q parameterizable for mini-tests."""
    import concourse.bass as bass
    from concourse import bacc
    import concourse.mybir as mybir
    import concourse.tile as tile
    from contextlib import ExitStack

    f32 = mybir.dt.float32
    f32r = mybir.dt.float32r

    n_sblk = s_keys // 512       # key production blocks
    n_tt = s_keys // 128         # key tiles (t)
    n_tgrp = max(1, n_tt // 8)   # AV groups of 8 t-tiles
    tt_per_grp = n_tt // n_tgrp
    n_qsb = s_q // 512           # query production blocks
    NS = s_q                     # resident query width (free dim in phase 2)
    n_ns = NS // 512             # N-splits for matmuls over queries

    nc = bacc.Bacc("TRN2", target_bir_lowering=False, debug=False)

    xT = nc.dram_tensor("xT", [D, s_keys], f32r, kind="ExternalInput").ap()
    xTq = nc.dram_tensor("xTq", [D, s_q], f32r, kind="ExternalInput").ap()
    WqT = nc.dram_tensor("WqT", [D, D], f32r, kind="ExternalInput").ap()
    WkT = nc.dram_tensor("WkT", [D, D], f32r, kind="ExternalInput").ap()
    WvT = nc.dram_tensor("WvT", [D, D], f32r, kind="ExternalInput").ap()
    SC2 = nc.dram_tensor("SC2", [ET, 128], f32, kind="ExternalInput").ap()   # ivl/(2pi)
    BQ2 = nc.dram_tensor("BQ2", [ET, 128], f32, kind="ExternalInput").ap()   # q bias/(2pi)
    BK2 = nc.dram_tensor("BK2", [ET, 128], f32, kind="ExternalInput").ap()   # k bias/(2pi)
    BV = nc.dram_tensor("BV", [1, D], f32r, kind="ExternalInput").ap()       # v bias row

    OT = nc.dram_tensor("OT", [D, s_q], f32, kind="ExternalOutput").ap()

    with tile.TileContext(nc) as tc, ExitStack() as top:
        # ---- DRAM intermediates (pool-tracked for RAW deps) ----
        dram = top.enter_context(tc.tile_pool(name="dram", bufs=1, space="DRAM"))
        CK_d = dram.tile([D, s_keys], f32r, tag="ck")
        SK_d = dram.tile([D, s_keys], f32r, tag="sk")
        CQ_d = dram.tile([D, s_q], f32r, tag="cq")
        SQ_d = dram.tile([D, s_q], f32r, tag="sq")
        V_d = dram.tile([s_keys, D], f32r, tag="v")

        # ---- constants ----
        cpool = top.enter_context(tc.tile_pool(name="consts", bufs=1))
        sc2 = [cpool.tile([128, 1], f32, tag=f"sc2_{i}", name=f"sc2_{i}") for i in range(ET)]
        bq2 = [cpool.tile([128, 1], f32, tag=f"bq2_{i}", name=f"bq2_{i}") for i in range(ET)]
        bk2 = [cpool.tile([128, 1], f32, tag=f"bk2_{i}", name=f"bk2_{i}") for i in range(ET)]
        for i in range(ET):
            nc.sync.dma_start(sc2[i][:], SC2[i : i + 1, :].rearrange("o p -> p o"))
            nc.sync.dma_start(bq2[i][:], BQ2[i : i + 1, :].rearrange("o p -> p o"))
            nc.sync.dma_start(bk2[i][:], BK2[i : i + 1, :].rearrange("o p -> p o"))
        bv = cpool.tile([1, D], f32r, tag="bv")
        nc.sync.dma_start(bv[:], BV[:])
        ones_f = cpool.tile([128, 2], f32, tag="ones_f")
        nc.vector.memset(ones_f[:], 1.0)
        ones_col = cpool.tile([128, 2], f32r, tag="ones_col")  # [K=128, M=2] rowsum lhsT
        nc.vector.tensor_copy(ones_col[:], ones_f[:])
        ones_rf = cpool.tile([1, 128], f32, tag="ones_rf")
        nc.vector.memset(ones_rf[:], 1.0)
        ones_row = cpool.tile([1, 128], f32r, tag="ones_row")  # [K=1, M=128] bcast lhsT
        nc.vector.tensor_copy(ones_row[:], ones_rf[:])

        # ---- weights resident ----
        wpool_ctx = tc.tile_pool(name="w", bufs=1)
        wpool = wpool_ctx.__enter__()
        wq = [wpool.tile([128, D], f32r, tag=f"wq{d}", name=f"wq{d}") for d in range(ET)]
        wk = [wpool.tile([128, D], f32r, tag=f"wk{d}", name=f"wk{d}") for d in range(ET)]
        wv = [wpool.tile([128, D], f32r, tag=f"wv{d}", name=f"wv{d}") for d in range(ET)]
        for d in range(ET):
            nc.sync.dma_start(wq[d][:], WqT[d * 128 : (d + 1) * 128, :])
            nc.sync.dma_start(wk[d][:], WkT[d * 128 : (d + 1) * 128, :])
            nc.sync.dma_start(wv[d][:], WvT[d * 128 : (d + 1) * 128, :])

        # ================= PHASE 1: projections + feature maps =================
        with tc.tile_pool(name="p1sb", bufs=2) as p1, \
             tc.tile_pool(name="p1chain", bufs=3) as pch, \
             tc.tile_pool(name="p1ps", bufs=4, space="PSUM") as pps:

            def feature_block(x_tiles, et, w_tiles, bias_tiles, c_stage, s_stage, width):
                """From x tiles produce cos/sin feature tiles [128, width] (f32r)."""
                ps = pps.tile([128, width], mybir.dt.float32, tag="proj")
                for d in range(ET):
                    nc.tensor.matmul(
                        ps[:], w_tiles[d][:, et * 128 : (et + 1) * 128],
                        x_tiles[d][:], start=(d == 0), stop=(d == ET - 1),
                    )
                r = pch.tile([128, width], mybir.dt.float32, tag="r")
                nc.scalar.activation(r[:], ps[:],
                                     mybir.ActivationFunctionType.Identity,
                                     scale=sc2[et][:], bias=bias_tiles[et][:])
                kk = pch.tile([128, width], mybir.dt.float32, tag="kk")
                nc.vector.tensor_scalar(kk[:], r[:], MAGIC, MAGIC,
                                        mybir.AluOpType.add, mybir.AluOpType.subtract)
                f = pch.tile([128, width], mybir.dt.float32, tag="f")
                nc.vector.scalar_tensor_tensor(f[:], kk[:], -1.0, r[:],
                                               mybir.AluOpType.mult,
                                               mybir.AluOpType.add)
                nc.scalar.activation(s_stage[:], f[:],
                                     mybir.ActivationFunctionType.Sin, scale=TWOPI)
                g = pch.tile([128, width], mybir.dt.float32, tag="g")
                nc.vector.add_range_wrap(g[:], f[:], 0.25, 0.5, 1.0)
                nc.scalar.activation(c_stage[:], g[:],
                                     mybir.ActivationFunctionType.Sin, scale=TWOPI)

            # --- Q features ---
            for qsb in range(n_qsb):
                xq = [p1.tile([128, 512], f32r, tag=f"xq{d}", name=f"xq{d}") for d in range(ET)]
                for d in range(ET):
                    nc.sync.dma_start(
                        xq[d][:], xTq[d * 128 : (d + 1) * 128, qsb * 512 : qsb * 512 + 512])
                for et in range(ET):
                    cstg = p1.tile([128, 512], f32r, tag="cstg")
                    sstg = p1.tile([128, 512], f32r, tag="sstg")
                    feature_block(xq, et, wq, bq2, cstg, sstg, 512)
                    nc.sync.dma_start(
                        CQ_d[et * 128 : (et + 1) * 128, qsb * 512 : qsb * 512 + 512],
                        cstg[:])
                    nc.sync.dma_start(
                        SQ_d[et * 128 : (et + 1) * 128, qsb * 512 : qsb * 512 + 512],
                        sstg[:])

            # --- K features + V ---
            for sblk in range(n_sblk):
                xk = [p1.tile([128, 512], f32r, tag=f"xk{d}", name=f"xk{d}") for d in range(ET)]
                for d in range(ET):
                    nc.sync.dma_start(
                        xk[d][:], xT[d * 128 : (d + 1) * 128, sblk * 512 : sblk * 512 + 512])
                for et in range(ET):
                    cstg = p1.tile([128, 512], f32r, tag="cstg")
                    sstg = p1.tile([128, 512], f32r, tag="sstg")
                    feature_block(xk, et, wk, bk2, cstg, sstg, 512)
                    nc.sync.dma_start(
                        CK_d[et * 128 : (et + 1) * 128, sblk * 512 : sblk * 512 + 512],
                        cstg[:])
                    nc.sync.dma_start(
                        SK_d[et * 128 : (et + 1) * 128, sblk * 512 : sblk * 512 + 512],
                        sstg[:])
                # V for the 4 t-tiles of this block, natural [t, dv] layout
                for ti in range(4):
                    t0 = sblk * 4 + ti
                    for dg in range(2):
                        psv = pps.tile([128, 512], mybir.dt.float32, tag="proj")
                        nc.tensor.matmul(psv[:], ones_row[:, :],
                                         bv[:, dg * 512 : dg * 512 + 512],
                                         start=True, stop=False)
                        for d in range(ET):
                            nc.tensor.matmul(
                                psv[:], xk[d][:, ti * 128 : (ti + 1) * 128],
                                wv[d][:, dg * 512 : dg * 512 + 512],
                                start=False, stop=(d == ET - 1))
                        vstg = p1.tile([128, 512], f32r, tag="vstg")
                        nc.vector.tensor_copy(vstg[:], psv[:])
                        nc.sync.dma_start(
                            V_d[t0 * 128 : (t0 + 1) * 128, dg * 512 : dg * 512 + 512],
                            vstg[:])

        wpool_ctx.__exit__(None, None, None)

        # ================= PHASE 2: scores + softmax + AV =================
        with tc.tile_pool(name="qres", bufs=1) as qres, \
             tc.tile_pool(name="p2sb", bufs=2) as p2, \
             tc.tile_pool(name="epool", bufs=tt_per_grp + 2) as epool, \
             tc.tile_pool(name="oacc", bufs=1) as oacc, \
             tc.tile_pool(name="p2ps", bufs=3, space="PSUM") as p2ps, \
             tc.tile_pool(name="rsps", bufs=1, space="PSUM") as rsps:

            cq = [qres.tile([128, NS], f32r, tag=f"cq{et}", name=f"cqr{et}") for et in range(ET)]
            sq = [qres.tile([128, NS], f32r, tag=f"sq{et}", name=f"sqr{et}") for et in range(ET)]
            for et in range(ET):
                nc.sync.dma_start(cq[et][:], CQ_d[et * 128 : (et + 1) * 128, :])
                nc.sync.dma_start(sq[et][:], SQ_d[et * 128 : (et + 1) * 128, :])

            o_ac = [oacc.tile([128, NS], mybir.dt.float32, tag=f"o{dt}", name=f"oac{dt}")
                    for dt in range(ET)]
            ps_rs = rsps.tile([2, NS], mybir.dt.float32, tag="rs")

            for tg in range(n_tgrp):
                e_tiles = []
                for ti in range(tt_per_grp):
                    tt = tg * tt_per_grp + ti
                    ck = p2.tile([128, D], f32r, tag="ck")
                    sk = p2.tile([128, D], f32r, tag="sk")
                    nc.sync.dma_start(
                        ck[:].rearrange("p (et t) -> p et t", et=ET),
                        CK_d[:, tt * 128 : (tt + 1) * 128]
                        .rearrange("(et p) t -> p et t", p=128))
                    nc.sync.dma_start(
                        sk[:].rearrange("p (et t) -> p et t", et=ET),
                        SK_d[:, tt * 128 : (tt + 1) * 128]
                        .rearrange("(et p) t -> p et t", p=128))
                    ps_sim = p2ps.tile([128, NS], mybir.dt.float32, tag="big")
                    for ns in range(n_ns):
                        sl = slice(ns * 512, ns * 512 + 512)
                        for et in range(ET):
                            nc.tensor.matmul(ps_sim[:, sl],
                                             ck[:, et * 128 : (et + 1) * 128],
                                             cq[et][:, sl],
                                             start=(et == 0), stop=False)
                        for et in range(ET):
                            nc.tensor.matmul(ps_sim[:, sl],
                                             sk[:, et * 128 : (et + 1) * 128],
                                             sq[et][:, sl],
                                             start=False, stop=(et == ET - 1))
                    et_t = epool.tile([128, NS], f32r, tag="e")
                    nc.scalar.activation(et_t[:], ps_sim[:],
                                         mybir.ActivationFunctionType.Exp,
                                         scale=INV_SQRT_D)
                    e_tiles.append((tt, et_t))
                    for ns in range(n_ns):
                        sl = slice(ns * 512, ns * 512 + 512)
                        nc.tensor.matmul(ps_rs[:, sl], ones_col[:], et_t[:, sl],
                                         start=(tt == 0), stop=(tt == n_tt - 1))
                # AV for this group
                for dt in range(ET):
                    ps_o = p2ps.tile([128, NS], mybir.dt.float32, tag="big")
                    for gi, (tt, et_t) in enumerate(e_tiles):
                        vt = p2.tile([128, 128], f32r, tag=f"v{dt}")
                        nc.sync.dma_start(
                            vt[:], V_d[tt * 128 : (tt + 1) * 128,
                                       dt * 128 : (dt + 1) * 128])
                        for ns in range(n_ns):
                            sl = slice(ns * 512, ns * 512 + 512)
                            nc.tensor.matmul(ps_o[:, sl], vt[:], et_t[:, sl],
                                             start=(gi == 0),
                                             stop=(gi == len(e_tiles) - 1))
                    if tg == 0:
                        nc.vector.tensor_copy(o_ac[dt][:], ps_o[:])
                    else:
                        nc.vector.tensor_tensor(o_ac[dt][:], ps_o[:], o_ac[dt][:],
                                                mybir.AluOpType.add)

            # normalize: recip of rowsum, broadcast via rank-1 matmul
            rs_sb = p2.tile([1, NS], mybir.dt.float32, tag="rs_sb")
            nc.vector.tensor_copy(rs_sb[:], ps_rs[:1, :])
            rec_f = p2.tile([1, NS], mybir.dt.float32, tag="rec_f")
            nc.vector.reciprocal(rec_f[:], rs_sb[:])
            rec = p2.tile([1, NS], f32r, tag="rec")
            nc.vector.tensor_copy(rec[:], rec_f[:])
            ps_bc = p2ps.tile([128, NS], mybir.dt.float32, tag="big")
            for ns in range(n_ns):
                sl = slice(ns * 512, ns * 512 + 512)
                nc.tensor.matmul(ps_bc[:, sl], ones_row[:], rec[:, sl],
                                 start=True, stop=True)
            bc = p2.tile([128, NS], mybir.dt.float32, tag="bc")
            nc.vector.tensor_copy(bc[:], ps_bc[:])
            for dt in range(ET):
                ot = p2.tile([128, NS], mybir.dt.float32, tag="ot")
                nc.vector.tensor_tensor(ot[:], o_ac[dt][:], bc[:],
                                        mybir.AluOpType.mult)
                nc.sync.dma_start(OT[dt * 128 : (dt + 1) * 128, :], ot[:])

    nc.compile()
    return nc


def _host_prep(x, Wq, bq, Wk, bk, Wv, bv, phase_bias):
    wavelengths = np.arange(1, D + 1, dtype=np.float32) * np.float32(2.0 * math.pi / D)
    inv_wl = (1.0 / (wavelengths + np.float32(1e-8))).astype(np.float32)
    sc2 = (inv_wl / TWOPI).astype(np.float32).reshape(ET, 128)
    bq2 = ((bq * inv_wl + phase_bias) / TWOPI).astype(np.float32).reshape(ET, 128)
    bk2 = ((bk * inv_wl + phase_bias) / TWOPI).astype(np.float32).reshape(ET, 128)
    WqT = np.ascontiguousarray(Wq.T).astype(np.float32)
    WkT = np.ascontiguousarray(Wk.T).astype(np.float32)
    WvT = np.ascontiguousarray(Wv.T).astype(np.float32)
    xT = [np.ascontiguousarray(x[b].T).astype(np.float32) for b in range(B)]
    return xT, WqT, WkT, WvT, sc2, bq2, bk2, bv.reshape(1, D).astype(np.float32)


def kernel(x, Wq, bq, Wk, bk, Wv, bv, phase_bias, _trace=False):
    from concourse.bass_utils import run_bass_kernel_spmd

    x = np.asarray(x, dtype=np.float32)
    xT, WqT, WkT, WvT, sc2, bq2, bk2, bvr = _host_prep(
        x, np.asarray(Wq, np.float32), np.asarray(bq, np.float32),
        np.asarray(Wk, np.float32), np.asarray(bk, np.float32),
        np.asarray(Wv, np.float32), np.asarray(bv, np.float32),
        np.asarray(phase_bias, np.float32))

    if "prog" not in _cache:
        _cache["prog"] = _build_program()
    nc = _cache["prog"]

    in_maps = []
    for c in range(NCORES):
        b, qb = c // 4, c % 4
        in_maps.append({
            "xT": xT[b],
            "xTq": np.ascontiguousarray(xT[b][:, qb * QBLK : (qb + 1) * QBLK]),
            "WqT": WqT, "WkT": WkT, "WvT": WvT,
            "SC2": sc2, "BQ2": bq2, "BK2": bk2, "BV": bvr,
        })
    res = run_bass_kernel_spmd(nc, in_maps, core_ids=list(range(NCORES)),
                               trace=_trace)
    out = np.empty((B, S, D), dtype=np.float32)
    for c in range(NCORES):
        b, qb = c // 4, c % 4
        out[b, qb * QBLK : (qb + 1) * QBLK, :] = res.results[c]["OT"].T
    if _trace:
        kernel.last_exec_time_ns = res.exec_time_ns
        kernel.last_result = res
    return out


# revision 17
# speedup vs baseline: 357.4475x; 357.4475x over previous
"""EulerAttention Trainium2 kernel.

Per-core sharding: core c in 0..7 -> (batch b = c // 4, query block qb = c % 4,
1024 queries each).  Each core computes K/V (+ feature maps) for its whole
batch, Q features for its query block, then flash-style scores/softmax/AV.

All matmuls run as float32r (full-rate fp32 with reduced multiplier mantissa);
e-tile 0 of the Q/K projections runs in full fp32 (the 1/wavelength scaling
amplifies its error ~200x more than the rest).  Feature maps cos/sin(theta)
use a turns-space range reduction (magic-number round + add_range_wrap)
feeding the ACT Sin table (valid +-pi).  Softmax runs without max-subtraction
(logits are bounded by sqrt(D)), rowsums via ones-matmul, normalization and
the V-bias fold happen on the output tiles.

kernel(**inputs) takes the full unsharded inputs from reference.setup_inputs()
and returns the full [B, S, D] output.
"""
import sys, math

sys.path.insert(0, "/opt/trn_rl_repo")

import numpy as np

B, S, D = 2, 4096, 1024
NCORES = 8
QBLK = S // 4          # queries per core
ET = D // 128          # number of 128-row e/d tiles (8)
MAGIC = float(1.5 * 2**23)
TWOPI = 2.0 * math.pi
INV_SQRT_D = 1.0 / math.sqrt(D)

_cache = {}


def _build_program(s_keys=S, s_q=QBLK, trace_sim=False, fp32_et0=True):
    """Build the SPMD bass program. s_keys/s_q parameterizable for mini-tests."""
    import concourse.bass as bass
    from concourse import bacc
    import concourse.mybir as mybir
    import concourse.tile as tile
    from contextlib import ExitStack

    f32 = mybir.dt.float32
    f32r = mybir.dt.float32r
    Act = mybir.ActivationFunctionType
    Alu = mybir.AluOpType

    n_sblk = s_keys // 512       # key production blocks (4 t-tiles each)
    n_tt = s_keys // 128         # key tiles (t)
    n_tgrp = max(1, n_tt // 8)   # AV groups of 8 t-tiles
    tt_per_grp = n_tt // n_tgrp
    n_qsb = s_q // 512           # query production blocks
    NS = s_q                     # resident query width (free dim in phase 2)
    n_ns = NS // 512             # N-splits for matmuls over queries
    n_eg = ET // 2               # et store groups of 2

    nc = bacc.Bacc("TRN2", target_bir_lowering=False, debug=False)

    xT = nc.dram_tensor("xT", [D, s_keys], f32, kind="ExternalInput").ap()
    xTq = nc.dram_tensor("xTq", [D, s_q], f32, kind="ExternalInput").ap()
    Wq0 = nc.dram_tensor("Wq0", [D, 128], f32, kind="ExternalInput").ap()
    Wk0 = nc.dram_tensor("Wk0", [D, 128], f32, kind="ExternalInput").ap()
    WqT = nc.dram_tensor("WqT", [D, D], f32r, kind="ExternalInput").ap()
    WkT = nc.dram_tensor("WkT", [D, D], f32r, kind="ExternalInput").ap()
    WvT = nc.dram_tensor("WvT", [D, D], f32r, kind="ExternalInput").ap()
    # packed per-partition constants: columns = (sc2 | bq2 | bk2 | bv) x ET
    CON = nc.dram_tensor("CON", [128, 4 * ET], f32, kind="ExternalInput").ap()

    OT = nc.dram_tensor("OT", [D, s_q], f32, kind="ExternalOutput").ap()

    with tile.TileContext(nc, trace_sim=trace_sim) as tc, ExitStack() as top:
        # ---- DRAM intermediates, split per block for fine-grained RAW deps ----
        dram = top.enter_context(tc.tile_pool(name="dram", bufs=1, space="DRAM"))
        CK_d = [dram.tile([D, 512], f32r, tag=f"ck{i}", name=f"ckd{i}")
                for i in range(n_sblk)]
        SK_d = [dram.tile([D, 512], f32r, tag=f"sk{i}", name=f"skd{i}")
                for i in range(n_sblk)]
        V_d = [dram.tile([512, D], f32r, tag=f"v{i}", name=f"vd{i}")
               for i in range(n_sblk)]

        # ---- constants (tiny, load first) ----
        cpool = top.enter_context(tc.tile_pool(name="consts", bufs=1))
        ctile = cpool.tile([128, 4 * ET], f32, tag="ctile")
        nc.sync.dma_start(ctile[:], CON[:])
        sc2 = [ctile[:, i : i + 1] for i in range(ET)]
        bq2 = [ctile[:, ET + i : ET + i + 1] for i in range(ET)]
        bk2 = [ctile[:, 2 * ET + i : 2 * ET + i + 1] for i in range(ET)]
        bvt = [ctile[:, 3 * ET + i : 3 * ET + i + 1] for i in range(ET)]
        ones_f = cpool.tile([128, 2], f32, tag="ones_f")
        nc.vector.memset(ones_f[:], 1.0)
        ones_col = cpool.tile([128, 2], f32r, tag="ones_col")  # [K=128, M=2] rowsum lhsT
        nc.vector.tensor_copy(ones_col[:], ones_f[:])
        ones_rf = cpool.tile([1, 128], f32, tag="ones_rf")
        nc.vector.memset(ones_rf[:], 1.0)
        ones_row = cpool.tile([1, 128], f32r, tag="ones_row")  # [K=1, M=128] bcast lhsT
        nc.vector.tensor_copy(ones_row[:], ones_rf[:])

        # ---- shared PSUM pool: proj (1 bank x2), big (2 banks x2), rs (2) ----
        psum = top.enter_context(tc.tile_pool(name="psum", bufs=1, space="PSUM"))

        # ---- resident Q feature maps, layout [128, (et, qsb, 512)]; written
        # directly by the Q-feature ACT ops, consumed by phase-2 matmuls ----
        qres = top.enter_context(tc.tile_pool(name="qres", bufs=1))
        cqa = qres.tile([128, ET * NS], f32r, tag="cqa")
        sqa = qres.tile([128, ET * NS], f32r, tag="sqa")

        # ---- weights: wq and wv share tiles (wq used only in the Q section);
        # fp32 copies of the e-tile-0 weight columns for the precise matmuls ----
        wpool_ctx = tc.tile_pool(name="w", bufs=1)
        wpool = wpool_ctx.__enter__()
        wsh = [wpool.tile([128, D], f32r, tag=f"wsh{d}", name=f"wsh{d}") for d in range(ET)]
        wk = [wpool.tile([128, D], f32r, tag=f"wk{d}", name=f"wk{d}") for d in range(ET)]
        w0 = [wpool.tile([128, 128], f32, tag=f"w0{d}", name=f"w0{d}") for d in range(ET)]
        if fp32_et0:
            for d in range(ET):
                nc.sync.dma_start(w0[d][:], Wq0[d * 128 : (d + 1) * 128, :])

        # ================= PHASE 1: projections + feature maps =================
        with tc.tile_pool(name="p1sb", bufs=2) as p1, \
             tc.tile_pool(name="p1chain", bufs=2) as pch:
            pps = psum

            def load_xblk(src_ap, col0):
                """One DMA: [1024, 512] dram slice -> fp32 block; DVE makes the
                rounded f32r copy for the fast-path matmuls (the DMA itself
                rounds when writing f32r, so the fp32 load preserves the full
                data for the e-tile-0 fp32 matmuls)."""
                b32 = p1.tile([128, ET * 512], f32, tag="xb32", name="xb32", bufs=1)
                nc.sync.dma_start(
                    b32[:].rearrange("p (d s) -> p d s", d=ET),
                    src_ap[:, col0 : col0 + 512].rearrange("(d p) s -> p d s", p=128))
                br = p1.tile([128, ET * 512], f32r, tag="xbr", name="xbr")
                nc.vector.tensor_copy(br[:], b32[:])
                return b32, br

            def feature_block(xb, et, w_tiles, bias_tiles, c_stage, s_stage):
                """Produce cos/sin feature tiles [128, 512] (f32r) for one e-tile."""
                xb32, xbr = xb
                ps = pps.tile([128, 512], f32, tag="proj", name="psf", bufs=2)
                for d in range(ET):
                    if fp32_et0 and et == 0:
                        lhs = w0[d][:]
                        rhs = xb32[:, d * 512 : (d + 1) * 512]
                    else:
                        lhs = w_tiles[d][:, et * 128 : (et + 1) * 128]
                        rhs = xbr[:, d * 512 : (d + 1) * 512]
                    nc.tensor.matmul(ps[:], lhs, rhs,
                                     start=(d == 0), stop=(d == ET - 1))
                r = pch.tile([128, 512], f32, tag="r", name="r")
                nc.scalar.activation(r[:], ps[:], Act.Identity,
                                     scale=sc2[et][:], bias=bias_tiles[et][:])
                kk = pch.tile([128, 512], f32, tag="kk", name="kk")
                nc.vector.tensor_scalar(kk[:], r[:], MAGIC, MAGIC, Alu.add, Alu.subtract)
                f = pch.tile([128, 512], f32, tag="f", name="f")
                nc.vector.scalar_tensor_tensor(f[:], kk[:], -1.0, r[:],
                                               Alu.mult, Alu.add)
                nc.scalar.activation(s_stage[:], f[:], Act.Sin, scale=TWOPI)
                g = pch.tile([128, 512], f32, tag="kk", name="g")
                nc.vector.add_range_wrap(g[:], f[:], 0.25, 0.5, 1.0)
                nc.scalar.activation(c_stage[:], g[:], Act.Sin, scale=TWOPI)

            def emit_k_features(xb, cdst, sdst):
                """K features: ACT output tiles stored directly per e-tile."""
                for et in range(ET):
                    cst = pch.tile([128, 512], f32r, tag="cst", name="cst")
                    sst = pch.tile([128, 512], f32r, tag="sst", name="sst")
                    feature_block(xb, et, wk, bk2, cst[:], sst[:])
                    nc.sync.dma_start(cdst[et * 128 : (et + 1) * 128, :], cst[:])
                    nc.sync.dma_start(sdst[et * 128 : (et + 1) * 128, :], sst[:])

            # --- Q features, written straight into the resident cqa/sqa ---
            xq_blocks = [load_xblk(xTq, 0)]
            for d in range(ET):
                nc.sync.dma_start(wsh[d][:], WqT[d * 128 : (d + 1) * 128, :])
            if n_qsb > 1:
                xq_blocks.append(load_xblk(xTq, 512))
            for d in range(ET):
                nc.sync.dma_start(wk[d][:], WkT[d * 128 : (d + 1) * 128, :])
            for qsb in range(n_qsb):
                xqb = xq_blocks[qsb]
                for et in range(ET):
                    feature_block(
                        xqb, et, wsh, bq2,
                        cqa[:, et * NS + qsb * 512 : et * NS + qsb * 512 + 512],
                        sqa[:, et * NS + qsb * 512 : et * NS + qsb * 512 + 512])

            if fp32_et0:
                for d in range(ET):
                    # w0k overwrites w0q (WAR dep handled by Tile)
                    nc.sync.dma_start(w0[d][:], Wk0[d * 128 : (d + 1) * 128, :])
            for d in range(ET):
                # wv overwrites the wq tiles (WAR dep handled by Tile)
                nc.sync.dma_start(wsh[d][:], WvT[d * 128 : (d + 1) * 128, :])

            # --- K features + V ---
            for sblk in range(n_sblk):
                xkb = load_xblk(xT, sblk * 512)
                emit_k_features(xkb, CK_d[sblk], SK_d[sblk])
                # V in natural [t, dv] layout, no bias (folded into output)
                for ti in range(4):
                    for dg in range(2):
                        psv = pps.tile([128, 512], f32, tag="proj", name="psv", bufs=2)
                        for d in range(ET):
                            nc.tensor.matmul(
                                psv[:], xkb[1][:, d * 512 + ti * 128 : d * 512 + (ti + 1) * 128],
                                wsh[d][:, dg * 512 : dg * 512 + 512],
                                start=(d == 0), stop=(d == ET - 1))
                        vstg = p1.tile([128, 512], f32r, tag="vstg", name="vstg")
                        nc.vector.tensor_copy(vstg[:], psv[:])
                        nc.sync.dma_start(
                            V_d[sblk][ti * 128 : (ti + 1) * 128,
                                      dg * 512 : (dg + 1) * 512], vstg[:])

        wpool_ctx.__exit__(None, None, None)

        # ================= PHASE 2: scores + softmax + AV =================
        with tc.tile_pool(name="p2sb", bufs=2) as p2, \
             tc.tile_pool(name="epool", bufs=tt_per_grp + 1) as epool, \
             tc.tile_pool(name="vpool", bufs=8) as vpool, \
             tc.tile_pool(name="oacc", bufs=1) as oacc:
            p2ps = psum
            rsps = psum

            def qslice(big, et, ns):
                return big[:, et * NS + ns * 512 : et * NS + ns * 512 + 512]

            o_ac = [oacc.tile([128, NS], f32, tag=f"o{dt}", name=f"oac{dt}")
                    for dt in range(ET)]
            ps_rs = rsps.tile([2, NS], f32, tag="rs", bufs=1)

            for tg in range(n_tgrp):
                e_tiles = []
                for ti in range(tt_per_grp):
                    tt = tg * tt_per_grp + ti
                    sb_i, loc = tt // 4, tt % 4
                    ck = p2.tile([128, D], f32r, tag="ck", name="ck")
                    sk = p2.tile([128, D], f32r, tag="sk", name="sk")
                    for dst, src in ((ck, CK_d[sb_i]), (sk, SK_d[sb_i])):
                        nc.sync.dma_start(
                            dst[:].rearrange("p (et t) -> p et t", et=ET),
                            src[:, loc * 128 : (loc + 1) * 128]
                            .rearrange("(et p) t -> p et t", p=128))
                    ps_sim = p2ps.tile([128, NS], f32, tag="big", name="ps_sim", bufs=2)
                    for ns in range(n_ns):
                        sl = slice(ns * 512, ns * 512 + 512)
                        for et in range(ET):
                            nc.tensor.matmul(ps_sim[:, sl],
                                             ck[:, et * 128 : (et + 1) * 128],
                                             qslice(cqa, et, ns),
                                             start=(et == 0), stop=False)
                        for et in range(ET):
                            nc.tensor.matmul(ps_sim[:, sl],
                                             sk[:, et * 128 : (et + 1) * 128],
                                             qslice(sqa, et, ns),
                                             start=False, stop=(et == ET - 1))
                    et_t = epool.tile([128, NS], f32r, tag="e", name="e")
                    nc.scalar.activation(et_t[:], ps_sim[:], Act.Exp, scale=INV_SQRT_D)
                    e_tiles.append((tt, et_t))
                    for ns in range(n_ns):
                        sl = slice(ns * 512, ns * 512 + 512)
                        nc.tensor.matmul(ps_rs[:, sl], ones_col[:], et_t[:, sl],
                                         start=(tt == 0), stop=(tt == n_tt - 1))
                # AV for this group
                for dg in range(2):
                    vts = []
                    for gi, (tt, _) in enumerate(e_tiles):
                        sb_i, loc = tt // 4, tt % 4
                        vt = vpool.tile([128, 512], f32r, tag="vt", name="vt")
                        nc.sync.dma_start(
                            vt[:], V_d[sb_i][loc * 128 : (loc + 1) * 128,
                                             dg * 512 : (dg + 1) * 512])
                        vts.append(vt)
                    for di in range(4):
                        dt = dg * 4 + di
                        ps_o = p2ps.tile([128, NS], f32, tag="big", name="ps_o", bufs=2)
                        for gi, (tt, et_t) in enumerate(e_tiles):
                            for ns in range(n_ns):
                                sl = slice(ns * 512, ns * 512 + 512)
                                nc.tensor.matmul(
                                    ps_o[:, sl], vts[gi][:, di * 128 : (di + 1) * 128],
                                    et_t[:, sl],
                                    start=(gi == 0), stop=(gi == len(e_tiles) - 1))
                        if tg == 0:
                            nc.vector.tensor_copy(o_ac[dt][:], ps_o[:])
                        else:
                            nc.vector.tensor_tensor(o_ac[dt][:], ps_o[:], o_ac[dt][:],
                                                    Alu.add)

            # normalize: recip of rowsum, broadcast via rank-1 matmul; + V bias
            rs_sb = p2.tile([1, NS], f32, tag="rs_sb")
            nc.vector.tensor_copy(rs_sb[:], ps_rs[:1, :])
            rec_f = p2.tile([1, NS], f32, tag="rec_f")
            nc.vector.reciprocal(rec_f[:], rs_sb[:])
            rec = p2.tile([1, NS], f32r, tag="rec")
            nc.vector.tensor_copy(rec[:], rec_f[:])
            ps_bc = p2ps.tile([128, NS], f32, tag="big", name="ps_bc", bufs=2)
            for ns in range(n_ns):
                sl = slice(ns * 512, ns * 512 + 512)
                nc.tensor.matmul(ps_bc[:, sl], ones_row[:], rec[:, sl],
                                 start=True, stop=True)
            bc = p2.tile([128, NS], f32, tag="bc")
            nc.vector.tensor_copy(bc[:], ps_bc[:])
            for dt in range(ET):
                on = p2.tile([128, NS], f32, tag="on", name="on")
                nc.vector.tensor_tensor(on[:], o_ac[dt][:], bc[:], Alu.mult)
                # per-partition V-bias add on ACT (idle at the tail)
                nc.scalar.activation(on[:], on[:], Act.Identity, bias=bvt[dt][:])
                nc.sync.dma_start(OT[dt * 128 : (dt + 1) * 128, :], on[:])

    nc.compile()
    return nc


def _host_prep(x, Wq, bq, Wk, bk, Wv, bv, phase_bias):
    wavelengths = np.arange(1, D + 1, dtype=np.float32) * np.float32(2.0 * math.pi / D)
    inv_wl = (np.float32(1.0) / (wavelengths + np.float32(1e-8))).astype(np.float32)
    sc2 = (inv_wl / TWOPI).astype(np.float32).reshape(ET, 128)
    bq2 = ((bq * inv_wl + phase_bias) / TWOPI).astype(np.float32).reshape(ET, 128)
    bk2 = ((bk * inv_wl + phase_bias) / TWOPI).astype(np.float32).reshape(ET, 128)
    WqT = np.ascontiguousarray(Wq.T).astype(np.float32)
    WkT = np.ascontiguousarray(Wk.T).astype(np.float32)
    WvT = np.ascontiguousarray(Wv.T).astype(np.float32)
    xT = [np.ascontiguousarray(x[b].T).astype(np.float32) for b in range(x.shape[0])]
    con = np.stack([sc2, bq2, bk2, bv.reshape(ET, 128).astype(np.float32)])
    # [4, ET, 128] -> [128, 4*ET] with column layout (kind, et)
    con = np.ascontiguousarray(con.reshape(4 * ET, 128).T).astype(np.float32)
    return xT, WqT, WkT, WvT, con


def kernel(x, Wq, bq, Wk, bk, Wv, bv, phase_bias, _trace=False):
    from concourse.bass_utils import run_bass_kernel_spmd

    x = np.asarray(x, dtype=np.float32)
    xT, WqT, WkT, WvT, con = _host_prep(
        x, np.asarray(Wq, np.float32), np.asarray(bq, np.float32),
        np.asarray(Wk, np.float32), np.asarray(bk, np.float32),
        np.asarray(Wv, np.float32), np.asarray(bv, np.float32),
        np.asarray(phase_bias, np.float32))

    if "prog" not in _cache:
        _cache["prog"] = _build_program()
    nc = _cache["prog"]

    in_maps = []
    for c in range(NCORES):
        b, qb = c // 4, c % 4
        in_maps.append({
            "xT": xT[b],
            "xTq": np.ascontiguousarray(xT[b][:, qb * QBLK : (qb + 1) * QBLK]),
            "WqT": WqT, "WkT": WkT, "WvT": WvT,
            "Wq0": np.ascontiguousarray(WqT[:, :128]),
            "Wk0": np.ascontiguousarray(WkT[:, :128]),
            "CON": con,
        })
    res = run_bass_kernel_spmd(nc, in_maps, core_ids=list(range(NCORES)),
                               trace=_trace)
    out = np.empty((B, S, D), dtype=np.float32)
    for c in range(NCORES):
        b, qb = c // 4, c % 4
        out[b, qb * QBLK : (qb + 1) * QBLK, :] = res.results[c]["OT"].T
    if _trace:
        kernel.last_exec_time_ns = res.exec_time_ns
        kernel.last_result = res
    return out
